# revision 25
# baseline (speedup 1.0000x reference)
"""Distributed attention kernel for 8 TRN2 NeuronCores.

Problem: B=2, S=2048, D=1024, H=16 heads (hd=64), no causal mask, no
scaling.  out = softmax((x@Wq) (x@Wk)^T) (x@Wv) @ Wp + biases.

Sharding: DP=2 over batch x TP=4 over heads.  Core c handles batch c//4
and heads 4*(c%4) .. 4*(c%4)+3.  Each core computes its 4 heads'
attention plus the partial c_proj (rows of w_proj for its heads), then a
chunked bf16 ReduceScatter(add) over its 4-core group yields each core's
512-row slice of the final output.  The host reassembles [2,2048,1024].

All matmuls run in bf16 (inputs converted host-side); accumulation f32.
Softmax skips the max-subtraction (scores are O(+-20), exp is safe in
f32): probs = exp(s) / sum exp(s); the denominator comes free as the
65th row of the PV matmul via an appended ones-column on V.
"""

import sys

if "/opt/trn_rl_repo" not in sys.path:
    sys.path.insert(0, "/opt/trn_rl_repo")

import numpy as np
import ml_dtypes

import concourse.bass as bass
import concourse.mybir as mybir
from concourse import bacc
from concourse.tile import TileContext
from concourse.bass_utils import run_bass_kernel_spmd

BF16 = mybir.dt.bfloat16
F32 = mybir.dt.float32

B, S, D = 2, 2048, 1024
H = 16
HD = 64
TP = 4  # tensor-parallel group size (cores per batch)
HPC = H // TP  # heads per core = 4
QC = HPC * HD  # q (or k or v) columns per core = 256
SQB = 512  # sq chunk (free dim of scores/pv matmuls)
NJ = S // SQB  # 4 chunks
NT = S // 128  # 16 sk tiles
NK = D // 128  # 8 contraction tiles for the projections
SO = S // TP  # 512 output rows per core

_CACHE = {}


def build():
    nc = bacc.Bacc(num_devices=8)

    xT_ext = nc.declare_dram_parameter("xT", [D, S], BF16, isOutput=False)
    wqkv_ext = nc.declare_dram_parameter("wqkv", [D, 3 * QC], BF16, isOutput=False)
    bqk_ext = nc.declare_dram_parameter("bqk", [2 * QC, 1], F32, isOutput=False)
    bv_ext = nc.declare_dram_parameter("bv", [1, QC], BF16, isOutput=False)
    wpa_ext = nc.declare_dram_parameter("wpa", [QC + 1, D], BF16, isOutput=False)
    out_ext = nc.declare_dram_parameter("out", [SO, D], F32, isOutput=True)

    partial = nc.dram_tensor("partial", [S, D], BF16)
    rs_out = [nc.dram_tensor(f"rs_out{j}", [SQB // TP, D], BF16) for j in range(NJ)]

    with TileContext(nc) as tc:
        with (
            tc.tile_pool(name="persist", bufs=1) as persist,
            tc.tile_pool(name="expt_pool", bufs=3) as expt_pool,
            tc.tile_pool(name="mm", bufs=3, space="PSUM") as mm_pool,
            tc.tile_pool(name="pv", bufs=2, space="PSUM") as pv_pool,
            tc.tile_pool(name="small", bufs=4) as small_pool,
            tc.tile_pool(name="ot", bufs=4) as ot_pool,
            tc.tile_pool(name="osb", bufs=6) as osb_pool,
        ):
            # ---- load persistent tiles ----
            # Interleave w/x loads in k order so the first qkv matmul
            # (k=0) can start as soon as the first pair lands.
            xt = []
            wt = []
            for k in range(NK):
                tw = persist.tile([128, 3 * QC], BF16, tag=f"wt{k}", name=f"wt{k}")
                nc.sync.dma_start(out=tw, in_=wqkv_ext[k * 128 : (k + 1) * 128, :])
                wt.append(tw)
                tx = persist.tile([128, S], BF16, tag=f"xt{k}", name=f"xt{k}")
                nc.sync.dma_start(out=tx, in_=xT_ext[k * 128 : (k + 1) * 128, :])
                xt.append(tx)
            wp = []
            for p in range(2):
                t = persist.tile([128, D], BF16, tag=f"wp{p}", name=f"wp{p}")
                nc.sync.dma_start(out=t, in_=wpa_ext[p * 128 : (p + 1) * 128, :])
                wp.append(t)
            wp_bias = persist.tile([1, D], BF16, tag="wpb", name="wpb")
            nc.sync.dma_start(out=wp_bias, in_=wpa_ext[2 * 128 : 2 * 128 + 1, :])
            bqk = []
            for k in range(4):
                t = persist.tile([128, 1], F32, tag=f"bqk{k}", name=f"bqk{k}")
                nc.sync.dma_start(out=t, in_=bqk_ext[k * 128 : (k + 1) * 128, :])
                bqk.append(t)
            bv = persist.tile([1, QC], BF16, tag="bv", name="bv")
            nc.sync.dma_start(out=bv, in_=bv_ext[:, :])
            ones_row = persist.tile([1, 128], BF16, tag="ones", name="ones")
            nc.vector.memset(ones_row, 1.0)

            # ---- QKV projection ----
            # q/k transposed layout: qk_sb[ct] [128, S], ct 0-1 = q cols,
            # ct 2-3 = k cols; head h lives on partitions (h%2)*64 of
            # tile h//2 (+2 for k).  Emit k first so attention can start
            # before the q tiles for later chunks are done.
            qk_sb = [
                persist.tile([128, S], BF16, tag=f"qk{ct}", name=f"qk{ct}")
                for ct in range(4)
            ]
            def qkv_col_tile(ct, ns):
                ps = mm_pool.tile([128, 2, SQB], F32, tag="mm", name="ps_qkv")
                for k in range(NK):
                    nc.tensor.matmul(
                        ps[:, 0, :],
                        wt[k][:, ct * 128 : (ct + 1) * 128],
                        xt[k][:, ns * SQB : (ns + 1) * SQB],
                        start=(k == 0),
                        stop=(k == NK - 1),
                    )
                nc.vector.tensor_scalar_add(
                    qk_sb[ct][:, ns * SQB : (ns + 1) * SQB], ps[:, 0, :], bqk[ct]
                )

            # k tiles first (attention needs the full kT), v next, then q
            # chunk-major so chunk 0's attention can start early.
            for ct in (2, 3):
                for ns in range(NJ):
                    qkv_col_tile(ct, ns)

            # v natural layout + ones column: v_sb[t] [128, HPC, 65];
            # [:, h, :64] = v for head h, [:, h, 64] = 1.0
            v_sb = []
            for t_i in range(NT):
                t = persist.tile(
                    [128, HPC, HD + 1], BF16, tag=f"v{t_i}", name=f"v{t_i}"
                )
                v_sb.append(t)
            for t_i in range(NT):
                psv = mm_pool.tile([128, 2, SQB], F32, tag="mm", name="ps_v")
                for k in range(NK):
                    nc.tensor.matmul(
                        psv[:, 0, 0:QC],
                        xt[k][:, t_i * 128 : (t_i + 1) * 128],
                        wt[k][:, 2 * QC : 3 * QC],
                        start=(k == 0),
                        stop=False,
                    )
                nc.tensor.matmul(psv[:, 0, 0:QC], ones_row, bv, start=False, stop=True)
                nc.vector.memset(v_sb[t_i][:, :, HD : HD + 1], 1.0)
                for h in range(HPC):
                    nc.vector.tensor_copy(
                        v_sb[t_i][:, h, 0:HD], psv[:, 0, h * HD : (h + 1) * HD]
                    )
            # q tiles for chunk 0 only; later chunks' q tiles are emitted
            # as PE filler inside the attention pipeline.
            for ct in (0, 1):
                qkv_col_tile(ct, 0)

            # ---- attention + c_proj + reduce-scatter, head-pipelined ----
            # Stage A(j,h): scores + exp.  Stage B(j,h): pv + normalize.
            # The attention inner loop is ACT(exp)-bound, so the in-order
            # PE queue must always hold ready work or the HAM clock gate
            # halves the PE clock.  After every stage we pop a few
            # "filler" groups (q projection for later chunks, c_proj
            # pieces for the previous chunk) that are ready to run.
            fillers = []

            def emit_fillers(n):
                for _ in range(n):
                    if fillers:
                        fillers.pop(0)()

            def stage_a(j, h):
                qslice = qk_sb[h // 2][
                    (h % 2) * HD : (h % 2) * HD + HD, j * SQB : (j + 1) * SQB
                ]
                krow = qk_sb[2 + h // 2][(h % 2) * HD : (h % 2) * HD + HD, :]
                expt = expt_pool.tile([128, NT, SQB], BF16, tag="expt", name="expt")
                for t2 in range(NT // 2):
                    ps_s = mm_pool.tile([128, 2, SQB], F32, tag="mm", name="ps_s")
                    for u in range(2):
                        t_i = 2 * t2 + u
                        nc.tensor.matmul(
                            ps_s[:, u, :],
                            krow[:, t_i * 128 : (t_i + 1) * 128],
                            qslice,
                            start=True,
                            stop=True,
                        )
                    nc.scalar.activation(
                        expt[:, 2 * t2 : 2 * t2 + 2, :],
                        ps_s,
                        mybir.ActivationFunctionType.Exp,
                    )
                return expt

            def stage_b(j, h, expt, om):
                pv = pv_pool.tile([HD + 1, SQB], F32, tag="pv", name=f"pv{h}")
                for t_i in range(NT):
                    nc.tensor.matmul(
                        pv,
                        v_sb[t_i][:, h, :],
                        expt[:, t_i, :],
                        start=(t_i == 0),
                        stop=(t_i == NT - 1),
                    )
                rz = small_pool.tile([1, SQB], F32, tag="rz", name="rz")
                nc.vector.reciprocal(rz, pv[HD : HD + 1, :])
                bc = small_pool.tile([HD, SQB], F32, tag="bc", name="bc")
                nc.gpsimd.partition_broadcast(bc, rz)
                if h % 2 == 0:
                    nc.vector.tensor_mul(om[h // 2][0:HD, :], pv[0:HD, :], bc)
                else:
                    o = ot_pool.tile([HD, SQB], BF16, tag="ot", name="ot")
                    nc.vector.tensor_mul(o, pv[0:HD, :], bc)
                    nc.sync.dma_start(out=om[h // 2][HD:128, :], in_=o)

            def cproj_piece(j, om, m, nch):
                pc = mm_pool.tile([128, 2, SQB], F32, tag="mm", name="pc")
                for p in range(2):
                    nc.tensor.matmul(
                        pc[:, 0, :],
                        om[p][:, m * 128 : (m + 1) * 128],
                        wp[p][:, nch * 512 : (nch + 1) * 512],
                        start=(p == 0),
                        stop=False,
                    )
                nc.tensor.matmul(
                    pc[:, 0, :],
                    ones_row,
                    wp_bias[:, nch * 512 : (nch + 1) * 512],
                    start=False,
                    stop=True,
                )
                osb = osb_pool.tile([128, 512], BF16, tag="osb", name="osb")
                nc.vector.tensor_copy(osb, pc[:, 0, :])
                nc.sync.dma_start(
                    out=partial[
                        j * SQB + m * 128 : j * SQB + (m + 1) * 128,
                        nch * 512 : (nch + 1) * 512,
                    ],
                    in_=osb,
                )

            def rs_piece(j):
                nc.gpsimd.collective_compute(
                    "ReduceScatter",
                    mybir.AluOpType.add,
                    replica_groups=[[0, 1, 2, 3], [4, 5, 6, 7]],
                    ins=[partial[j * SQB : (j + 1) * SQB, :]],
                    outs=[rs_out[j].ap()],
                )
                rcast = osb_pool.tile([128, D], BF16, tag="rcast", name="rcast")
                nc.sync.dma_start(out=rcast, in_=rs_out[j][:, :])
                rf32 = osb_pool.tile([128, D], F32, tag="rf32", name="rf32")
                nc.vector.tensor_copy(rf32, rcast)
                nc.sync.dma_start(out=out_ext[j * 128 : (j + 1) * 128, :], in_=rf32)

            def enqueue_cproj(j, om):
                for m in range(SQB // 128):
                    for nch in range(2):
                        fillers.append(
                            lambda j=j, om=om, m=m, nch=nch: cproj_piece(j, om, m, nch)
                        )
                fillers.append(lambda j=j: rs_piece(j))

            om_of = {}
            for j in range(NJ):
                om_of[j] = [
                    ot_pool.tile([128, SQB], BF16, tag="om", name=f"om{p}")
                    for p in range(2)
                ]
                prev = None
                for h in range(HPC):
                    expt = stage_a(j, h)
                    if prev is not None:
                        stage_b(j, prev[0], prev[1], om_of[j])
                    # q tiles for the NEXT chunk are emitted at fixed
                    # positions (h=0,1) so they always precede that
                    # chunk's stage_a in program order; c_proj/RS pieces
                    # fill the remaining slots.
                    if h < 2 and j + 1 < NJ:
                        qkv_col_tile(h, j + 1)
                        emit_fillers(1)
                    else:
                        emit_fillers(2)
                    prev = (h, expt)
                stage_b(j, prev[0], prev[1], om_of[j])
                enqueue_cproj(j, om_of[j])
                emit_fillers(2)
            emit_fillers(len(fillers))

    nc.compile()
    return nc


def make_in_maps(x, w_attn, b_attn, w_proj, b_proj):
    bf = ml_dtypes.bfloat16
    in_maps = []
    for c in range(8):
        b = c // TP
        g = c % TP
        cs = slice(g * QC, (g + 1) * QC)
        xT = np.ascontiguousarray(x[b].T).astype(bf)
        wqkv = np.concatenate(
            [w_attn[:, cs], w_attn[:, D:][:, cs], w_attn[:, 2 * D :][:, cs]], axis=1
        ).astype(bf)
        bqk = np.concatenate([b_attn[cs], b_attn[D:][cs]]).reshape(2 * QC, 1)
        bqk = np.ascontiguousarray(bqk, dtype=np.float32)
        bv = np.ascontiguousarray(b_attn[2 * D :][cs].reshape(1, QC).astype(bf))
        wpa = np.concatenate(
            [w_proj[cs, :], (b_proj / TP).reshape(1, D)], axis=0
        ).astype(bf)
        in_maps.append({"xT": xT, "wqkv": wqkv, "bqk": bqk, "bv": bv, "wpa": wpa})
    return in_maps


def assemble(results):
    # Chunk j's reduce-scatter gives core (group rank g) rows
    # j*SQB + g*128 .. +128; the kernel writes them to out rows j*128..,
    # so core c's "out" holds rows {j*SQB + g*128 + r} for j in 0..3.
    out = np.empty((B, S, D), np.float32)
    for c in range(8):
        b = c // TP
        g = c % TP
        o = results[c]["out"]
        for j in range(NJ):
            out[b, j * SQB + g * 128 : j * SQB + (g + 1) * 128, :] = o[
                j * 128 : (j + 1) * 128
            ]
    return out


def kernel(x, w_attn, b_attn, w_proj, b_proj):
    x = np.asarray(x, dtype=np.float32)
    w_attn = np.asarray(w_attn, dtype=np.float32)
    b_attn = np.asarray(b_attn, dtype=np.float32)
    w_proj = np.asarray(w_proj, dtype=np.float32)
    b_proj = np.asarray(b_proj, dtype=np.float32)
    if "nc" not in _CACHE:
        _CACHE["nc"] = build()
    nc = _CACHE["nc"]
    in_maps = make_in_maps(x, w_attn, b_attn, w_proj, b_proj)
    res = run_bass_kernel_spmd(nc, in_maps, core_ids=list(range(8)))
    return assemble(res.results)


# revision 31
# speedup vs baseline: 1.0291x; 1.0291x over previous
"""Distributed attention kernel for 8 TRN2 NeuronCores.

Problem: B=2, S=2048, D=1024, H=16 heads (hd=64), no causal mask, no
scaling.  out = softmax((x@Wq) (x@Wk)^T) (x@Wv) @ Wp + biases.

Sharding: DP=2 over batch x TP=4 over heads.  Core c handles batch c//4
and heads 4*(c%4) .. 4*(c%4)+3.  Each core computes its 4 heads'
attention plus the partial c_proj (rows of w_proj for its heads), then a
chunked bf16 ReduceScatter(add) over its 4-core group yields each core's
512-row slice of the final output.  The host reassembles [2,2048,1024].

All matmuls run in bf16 (inputs converted host-side); accumulation f32.
Softmax skips the max-subtraction (scores are O(+-20), exp is safe in
f32): probs = exp(s) / sum exp(s); the denominator comes free as the
65th row of the PV matmul via an appended ones-column on V.
"""

import sys

if "/opt/trn_rl_repo" not in sys.path:
    sys.path.insert(0, "/opt/trn_rl_repo")

import numpy as np
import ml_dtypes

import concourse.bass as bass
import concourse.mybir as mybir
from concourse import bacc
from concourse.tile import TileContext
from concourse.bass_utils import run_bass_kernel_spmd

BF16 = mybir.dt.bfloat16
F32 = mybir.dt.float32

B, S, D = 2, 2048, 1024
H = 16
HD = 64
TP = 4  # tensor-parallel group size (cores per batch)
HPC = H // TP  # heads per core = 4
QC = HPC * HD  # q (or k or v) columns per core = 256
SQB = 512  # sq chunk (free dim of scores/pv matmuls)
NJ = S // SQB  # 4 chunks
NT = S // 128  # 16 sk tiles
NK = D // 128  # 8 contraction tiles for the projections
SO = S // TP  # 512 output rows per core

_CACHE = {}


def build():
    nc = bacc.Bacc(num_devices=8)

    xT_ext = nc.declare_dram_parameter("xT", [D, S], BF16, isOutput=False)
    wqkv_ext = nc.declare_dram_parameter("wqkv", [D, 3 * QC], BF16, isOutput=False)
    bqk_ext = nc.declare_dram_parameter("bqk", [2 * QC, 1], F32, isOutput=False)
    bv_ext = nc.declare_dram_parameter("bv", [1, QC], BF16, isOutput=False)
    wpa_ext = nc.declare_dram_parameter("wpa", [QC + 1, D], BF16, isOutput=False)
    out_ext = nc.declare_dram_parameter("out", [SO, D], BF16, isOutput=True)

    partial = nc.dram_tensor("partial", [S, D], BF16)
    rs_out = [nc.dram_tensor(f"rs_out{j}", [SQB // TP, D], BF16) for j in range(NJ)]

    with TileContext(nc) as tc:
        with (
            tc.tile_pool(name="persist", bufs=1) as persist,
            tc.tile_pool(name="expt_pool", bufs=3) as expt_pool,
            tc.tile_pool(name="mm", bufs=3, space="PSUM") as mm_pool,
            tc.tile_pool(name="pv", bufs=2, space="PSUM") as pv_pool,
            tc.tile_pool(name="small", bufs=4) as small_pool,
            tc.tile_pool(name="ot", bufs=4) as ot_pool,
            tc.tile_pool(name="osb", bufs=6) as osb_pool,
        ):
            # ---- load persistent tiles ----
            # Interleave w/x loads in k order so the first qkv matmul
            # (k=0) can start as soon as the first pair lands.
            xt = []
            wt = []
            for k in range(NK):
                tw = persist.tile([128, 3 * QC], BF16, tag=f"wt{k}", name=f"wt{k}")
                nc.sync.dma_start(out=tw, in_=wqkv_ext[k * 128 : (k + 1) * 128, :])
                wt.append(tw)
                tx = persist.tile([128, S], BF16, tag=f"xt{k}", name=f"xt{k}")
                nc.sync.dma_start(out=tx, in_=xT_ext[k * 128 : (k + 1) * 128, :])
                xt.append(tx)
            wp = []
            for p in range(2):
                t = persist.tile([128, D], BF16, tag=f"wp{p}", name=f"wp{p}")
                nc.sync.dma_start(out=t, in_=wpa_ext[p * 128 : (p + 1) * 128, :])
                wp.append(t)
            wp_bias = persist.tile([1, D], BF16, tag="wpb", name="wpb")
            nc.sync.dma_start(out=wp_bias, in_=wpa_ext[2 * 128 : 2 * 128 + 1, :])
            bqk = []
            for k in range(4):
                t = persist.tile([128, 1], F32, tag=f"bqk{k}", name=f"bqk{k}")
                nc.sync.dma_start(out=t, in_=bqk_ext[k * 128 : (k + 1) * 128, :])
                bqk.append(t)
            bv = persist.tile([1, QC], BF16, tag="bv", name="bv")
            nc.sync.dma_start(out=bv, in_=bv_ext[:, :])
            ones_row = persist.tile([1, 128], BF16, tag="ones", name="ones")
            nc.vector.memset(ones_row, 1.0)

            # ---- QKV projection ----
            # q/k transposed layout: qk_sb[ct] [128, S], ct 0-1 = q cols,
            # ct 2-3 = k cols; head h lives on partitions (h%2)*64 of
            # tile h//2 (+2 for k).  Emit k first so attention can start
            # before the q tiles for later chunks are done.
            qk_sb = [
                persist.tile([128, S], BF16, tag=f"qk{ct}", name=f"qk{ct}")
                for ct in range(4)
            ]
            def qkv_col_tile(ct, ns):
                ps = mm_pool.tile([128, 2, SQB], F32, tag="mm", name="ps_qkv")
                for k in range(NK):
                    nc.tensor.matmul(
                        ps[:, 0, :],
                        wt[k][:, ct * 128 : (ct + 1) * 128],
                        xt[k][:, ns * SQB : (ns + 1) * SQB],
                        start=(k == 0),
                        stop=(k == NK - 1),
                    )
                nc.vector.tensor_scalar_add(
                    qk_sb[ct][:, ns * SQB : (ns + 1) * SQB], ps[:, 0, :], bqk[ct]
                )

            # k tiles first (attention needs the full kT), v next, then q
            # chunk-major so chunk 0's attention can start early.
            for ct in (2, 3):
                for ns in range(NJ):
                    qkv_col_tile(ct, ns)

            # v natural layout + ones column: v_sb[t] [128, HPC, 65];
            # [:, h, :64] = v for head h, [:, h, 64] = 1.0
            v_sb = []
            for t_i in range(NT):
                t = persist.tile(
                    [128, HPC, HD + 1], BF16, tag=f"v{t_i}", name=f"v{t_i}"
                )
                v_sb.append(t)
            for t_i in range(NT):
                psv = mm_pool.tile([128, 2, SQB], F32, tag="mm", name="ps_v")
                for k in range(NK):
                    nc.tensor.matmul(
                        psv[:, 0, 0:QC],
                        xt[k][:, t_i * 128 : (t_i + 1) * 128],
                        wt[k][:, 2 * QC : 3 * QC],
                        start=(k == 0),
                        stop=False,
                    )
                nc.tensor.matmul(psv[:, 0, 0:QC], ones_row, bv, start=False, stop=True)
                nc.vector.memset(v_sb[t_i][:, :, HD : HD + 1], 1.0)
                for h in range(HPC):
                    nc.vector.tensor_copy(
                        v_sb[t_i][:, h, 0:HD], psv[:, 0, h * HD : (h + 1) * HD]
                    )
            # q tiles for chunk 0 only; later chunks' q tiles are emitted
            # as PE filler inside the attention pipeline.
            for ct in (0, 1):
                qkv_col_tile(ct, 0)

            # ---- attention + c_proj + reduce-scatter, head-pipelined ----
            # Stage A(j,h): scores + exp.  Stage B(j,h): pv + normalize.
            # The attention inner loop is ACT(exp)-bound, so the in-order
            # PE queue must always hold ready work or the HAM clock gate
            # halves the PE clock.  After every stage we pop a few
            # "filler" groups (q projection for later chunks, c_proj
            # pieces for the previous chunk) that are ready to run.
            fillers = []

            def emit_fillers(n):
                for _ in range(n):
                    if fillers:
                        fillers.pop(0)()

            def normalize(h, pv, om):
                rz = small_pool.tile([1, SQB], F32, tag="rz", name="rz")
                nc.vector.reciprocal(rz, pv[HD : HD + 1, :])
                bc = small_pool.tile([HD, SQB], F32, tag="bc", name="bc")
                nc.gpsimd.partition_broadcast(bc, rz)
                if h % 2 == 0:
                    nc.vector.tensor_mul(om[h // 2][0:HD, :], pv[0:HD, :], bc)
                else:
                    o = ot_pool.tile([HD, SQB], BF16, tag="ot", name="ot")
                    nc.vector.tensor_mul(o, pv[0:HD, :], bc)
                    nc.sync.dma_start(out=om[h // 2][HD:128, :], in_=o)

            def stage_ab(j, h, prev):
                """Scores+exp for (j,h) interleaved with the pv matmuls of
                the previous head `prev` = (pj, ph, expt, om) or None.
                The pv matmuls need no scores-psum slot, so the PE always
                has ready work while ACT drains the scores banks."""
                expt = None
                if h is not None:
                    qslice = qk_sb[h // 2][
                        (h % 2) * HD : (h % 2) * HD + HD, j * SQB : (j + 1) * SQB
                    ]
                    krow = qk_sb[2 + h // 2][(h % 2) * HD : (h % 2) * HD + HD, :]
                    expt = expt_pool.tile(
                        [128, NT, SQB], BF16, tag="expt", name="expt"
                    )
                pvp = None
                if prev is not None:
                    pj, ph, pexpt, pom = prev
                    pvp = pv_pool.tile([HD + 1, SQB], F32, tag="pv", name="pv")
                for t2 in range(NT // 2):
                    if h is not None:
                        ps_s = mm_pool.tile(
                            [128, 2, SQB], F32, tag="mm", name="ps_s"
                        )
                        for u in range(2):
                            t_i = 2 * t2 + u
                            nc.tensor.matmul(
                                ps_s[:, u, :],
                                krow[:, t_i * 128 : (t_i + 1) * 128],
                                qslice,
                                start=True,
                                stop=True,
                            )
                        nc.scalar.activation(
                            expt[:, 2 * t2 : 2 * t2 + 2, :],
                            ps_s,
                            mybir.ActivationFunctionType.Exp,
                        )
                    if prev is not None:
                        for u in range(2):
                            t_i = 2 * t2 + u
                            nc.tensor.matmul(
                                pvp,
                                v_sb[t_i][:, ph, :],
                                pexpt[:, t_i, :],
                                start=(t_i == 0),
                                stop=(t_i == NT - 1),
                            )
                if prev is not None:
                    normalize(ph, pvp, pom)
                return expt

            def cproj_piece(j, om, m, nch):
                pc = mm_pool.tile([128, 2, SQB], F32, tag="mm", name="pc")
                for p in range(2):
                    nc.tensor.matmul(
                        pc[:, 0, :],
                        om[p][:, m * 128 : (m + 1) * 128],
                        wp[p][:, nch * 512 : (nch + 1) * 512],
                        start=(p == 0),
                        stop=False,
                    )
                nc.tensor.matmul(
                    pc[:, 0, :],
                    ones_row,
                    wp_bias[:, nch * 512 : (nch + 1) * 512],
                    start=False,
                    stop=True,
                )
                osb = osb_pool.tile([128, 512], BF16, tag="osb", name="osb")
                nc.vector.tensor_copy(osb, pc[:, 0, :])
                nc.sync.dma_start(
                    out=partial[
                        j * SQB + m * 128 : j * SQB + (m + 1) * 128,
                        nch * 512 : (nch + 1) * 512,
                    ],
                    in_=osb,
                )

            def rs_piece(j):
                nc.gpsimd.collective_compute(
                    "ReduceScatter",
                    mybir.AluOpType.add,
                    replica_groups=[[0, 1, 2, 3], [4, 5, 6, 7]],
                    ins=[partial[j * SQB : (j + 1) * SQB, :]],
                    outs=[rs_out[j].ap()],
                )
                nc.sync.dma_start(
                    out=out_ext[j * 128 : (j + 1) * 128, :], in_=rs_out[j][:, :]
                )

            def enqueue_cproj(j, om):
                for m in range(SQB // 128):
                    for nch in range(2):
                        fillers.append(
                            lambda j=j, om=om, m=m, nch=nch: cproj_piece(j, om, m, nch)
                        )
                fillers.append(lambda j=j: rs_piece(j))

            om_of = {}
            prev = None  # (pj, ph, expt, om) pending pv/normalize
            for j in range(NJ):
                om_of[j] = [
                    ot_pool.tile([128, SQB], BF16, tag="om", name=f"om{p}")
                    for p in range(2)
                ]
                for h in range(HPC):
                    expt = stage_ab(j, h, prev)
                    prev = (j, h, expt, om_of[j])
                    # q tiles for the NEXT chunk at fixed positions so
                    # they always precede that chunk's scores in program
                    # order; c_proj/RS pieces fill the remaining slots.
                    if h < 2 and j + 1 < NJ:
                        qkv_col_tile(h, j + 1)
                        emit_fillers(1)
                    else:
                        emit_fillers(2)
                    if j > 0 and h == 0:
                        enqueue_cproj(j - 1, om_of[j - 1])
            # drain: pv/normalize for the last head, last cproj/RS
            stage_ab(None, None, prev)
            enqueue_cproj(NJ - 1, om_of[NJ - 1])
            emit_fillers(len(fillers))

    nc.compile()
    return nc


def make_in_maps(x, w_attn, b_attn, w_proj, b_proj):
    bf = ml_dtypes.bfloat16
    in_maps = []
    for c in range(8):
        b = c // TP
        g = c % TP
        cs = slice(g * QC, (g + 1) * QC)
        xT = np.ascontiguousarray(x[b].T).astype(bf)
        wqkv = np.concatenate(
            [w_attn[:, cs], w_attn[:, D:][:, cs], w_attn[:, 2 * D :][:, cs]], axis=1
        ).astype(bf)
        bqk = np.concatenate([b_attn[cs], b_attn[D:][cs]]).reshape(2 * QC, 1)
        bqk = np.ascontiguousarray(bqk, dtype=np.float32)
        bv = np.ascontiguousarray(b_attn[2 * D :][cs].reshape(1, QC).astype(bf))
        wpa = np.concatenate(
            [w_proj[cs, :], (b_proj / TP).reshape(1, D)], axis=0
        ).astype(bf)
        in_maps.append({"xT": xT, "wqkv": wqkv, "bqk": bqk, "bv": bv, "wpa": wpa})
    return in_maps


def assemble(results):
    # Chunk j's reduce-scatter gives core (group rank g) rows
    # j*SQB + g*128 .. +128; the kernel writes them to out rows j*128..,
    # so core c's "out" holds rows {j*SQB + g*128 + r} for j in 0..3.
    out = np.empty((B, S, D), np.float32)
    for c in range(8):
        b = c // TP
        g = c % TP
        o = np.asarray(results[c]["out"]).astype(np.float32)
        for j in range(NJ):
            out[b, j * SQB + g * 128 : j * SQB + (g + 1) * 128, :] = o[
                j * 128 : (j + 1) * 128
            ]
    return out


def kernel(x, w_attn, b_attn, w_proj, b_proj):
    x = np.asarray(x, dtype=np.float32)
    w_attn = np.asarray(w_attn, dtype=np.float32)
    b_attn = np.asarray(b_attn, dtype=np.float32)
    w_proj = np.asarray(w_proj, dtype=np.float32)
    b_proj = np.asarray(b_proj, dtype=np.float32)
    if "nc" not in _CACHE:
        _CACHE["nc"] = build()
    nc = _CACHE["nc"]
    in_maps = make_in_maps(x, w_attn, b_attn, w_proj, b_proj)
    res = run_bass_kernel_spmd(nc, in_maps, core_ids=list(range(8)))
    return assemble(res.results)
